# revision 6
# baseline (speedup 1.0000x reference)
"""Trainium2 Bass kernel for nn_LocalAttention (sparse window attention).

Math (per sample b):
  x_b      = tanh(s_b @ W_omega_w + b_omega_w) @ W_omega_v + b_omega_v
  pt_b     = 1919 * sigmoid(x_b) + 64
  base_b   = floor(pt_b)
  idx      = base_b + [-64..64]                       (129 window rows)
  enc_w    = enc_output[b, idx, :]                    (gathered on device)
  enc_w_t  = enc_w @ W_attn + b_attn                  -> output 2
  alpha    = enc_w_t . s_b  (== enc_w . (W_attn@s_b) + b_attn.s_b)
  gauss    = exp(-((w-64-frac_b)^2)/2048)
  out1     = softmax(alpha * gauss)                   -> output 1

Sharding: data-parallel over batch; 16 samples per core on 8 cores.
The window rows are fetched with indirect (gather) DMA so only ~67MB of
enc_output is ever read instead of the full 1GB.
"""

import numpy as np

import concourse.bass as bass
import concourse.bacc as bacc
import concourse.mybir as mybir
import concourse.tile as tile
from concourse.bass import IndirectOffsetOnAxis, ts
from concourse.bass_utils import run_bass_kernel_spmd
from concourse.masks import make_identity

F32 = mybir.dt.float32
I32 = mybir.dt.int32

B = 128
NC = 8
BPC = B // NC          # 16 samples per core
S_LEN = 2048
H = 1024
D = 64
W = 2 * D + 1          # 129
KC = H // 128          # 8 contraction chunks
SIG_SCALE = float(S_LEN - 2 * D - 1)  # 1919

_CACHE = {}


def _build():
    nc = bacc.Bacc()
    enc_h = nc.declare_dram_parameter("enc", [BPC * S_LEN, H], F32, isOutput=False)
    sT_h = nc.declare_dram_parameter("st", [H, BPC], F32, isOutput=False)
    wow_h = nc.declare_dram_parameter("wow", [H, H], F32, isOutput=False)
    bow_h = nc.declare_dram_parameter("bow", [H], F32, isOutput=False)
    wv_h = nc.declare_dram_parameter("wv", [H], F32, isOutput=False)
    bv_h = nc.declare_dram_parameter("bv", [1], F32, isOutput=False)
    wa_h = nc.declare_dram_parameter("wa", [H, H], F32, isOutput=False)
    wat_h = nc.declare_dram_parameter("wat", [H, H], F32, isOutput=False)
    ba_h = nc.declare_dram_parameter("ba", [H], F32, isOutput=False)
    encwt_h = nc.declare_dram_parameter("encwt", [BPC * W, H], F32, isOutput=True)
    soft_h = nc.declare_dram_parameter("soft", [BPC, W], F32, isOutput=True)

    from contextlib import ExitStack

    with tile.TileContext(nc) as tc, ExitStack() as ctx:
        consts = ctx.enter_context(tc.tile_pool(name="consts", bufs=1))
        wpool = ctx.enter_context(tc.tile_pool(name="weights", bufs=1))
        pool_a = ctx.enter_context(tc.tile_pool(name="gather", bufs=3))
        pool_at = ctx.enter_context(tc.tile_pool(name="at", bufs=2))
        pool_osb = ctx.enter_context(tc.tile_pool(name="osb", bufs=3))
        tail = ctx.enter_context(tc.tile_pool(name="tail", bufs=1))
        psum_o = ctx.enter_context(tc.tile_pool(name="pso", bufs=2, space="PSUM"))
        psum_tr = ctx.enter_context(tc.tile_pool(name="pstr", bufs=2, space="PSUM"))
        psum_sm = ctx.enter_context(tc.tile_pool(name="pssm", bufs=2, space="PSUM"))

        if True:
            # ---- constants (on-device) ----
            i128 = consts.tile([128, 128], F32)
            make_identity(nc, i128[:])
            ones = consts.tile([1, 128], F32)
            nc.vector.memset(ones[:], 1.0)
            ro_i = consts.tile([BPC, 1], I32)
            nc.gpsimd.iota(ro_i[:], pattern=[[0, 1]], base=-D,
                           channel_multiplier=S_LEN)
            ro_f = consts.tile([BPC, 1], F32)
            nc.vector.tensor_copy(ro_f[:], ro_i[:])
            wi_i = consts.tile([128, 1], I32)
            nc.gpsimd.iota(wi_i[:], pattern=[[0, 1]], base=0, channel_multiplier=1)
            wi_f = consts.tile([128, 1], F32)
            nc.vector.tensor_copy(wi_f[:], wi_i[:])
            wm_i = consts.tile([BPC, W], I32)
            nc.gpsimd.iota(wm_i[:], pattern=[[1, W]], base=-D, channel_multiplier=0)
            wm_f = consts.tile([BPC, W], F32)
            nc.vector.tensor_copy(wm_f[:], wm_i[:])

            # ---- resident weights ----
            wa_sb = wpool.tile([128, KC, H], F32)
            nc.sync.dma_start(out=wa_sb[:], in_=wa_h[:, :].rearrange("(k p) n -> p k n", p=128))
            wat_sb = wpool.tile([128, KC, H], F32)
            nc.sync.dma_start(out=wat_sb[:], in_=wat_h[:, :].rearrange("(k p) n -> p k n", p=128))
            wow_sb = wpool.tile([128, KC, H], F32)
            nc.sync.dma_start(out=wow_sb[:], in_=wow_h[:, :].rearrange("(k p) n -> p k n", p=128))
            sT_sb = wpool.tile([128, KC, BPC], F32)
            nc.sync.dma_start(out=sT_sb[:], in_=sT_h[:, :].rearrange("(k p) b -> p k b", p=128))
            ba_sb = wpool.tile([1, H], F32)
            nc.sync.dma_start(out=ba_sb[:], in_=ba_h[:].rearrange("(o n) -> o n", o=1))
            baT_sb = wpool.tile([128, KC], F32)
            nc.sync.dma_start(out=baT_sb[:], in_=ba_h[:].rearrange("(k p) -> p k", p=128))
            bow_sb = wpool.tile([1, H], F32)
            nc.sync.dma_start(out=bow_sb[:], in_=bow_h[:].rearrange("(o n) -> o n", o=1))
            wvb_sb = wpool.tile([BPC, H], F32)
            nc.sync.dma_start(out=wvb_sb[:], in_=bass.AP(
                tensor=wv_h[:].tensor, offset=wv_h[:].offset, ap=[[0, BPC], [1, H]]))
            bv_sb = wpool.tile([BPC, 1], F32)
            nc.sync.dma_start(out=bv_sb[:], in_=bass.AP(
                tensor=bv_h[:].tensor, offset=bv_h[:].offset, ap=[[0, BPC], [1, 1]]))

            # ---- stage A: pt / base / gather indices ----
            out1_ps = psum_o.tile([BPC, H], F32, tag="o")
            for k in range(KC):
                lhsT = sT_sb[:, k, :]
                nc.tensor.matmul(out1_ps[:, 0:512], lhsT, wow_sb[:, k, 0:512],
                                 start=(k == 0), stop=False)
                nc.tensor.matmul(out1_ps[:, 512:1024], lhsT, wow_sb[:, k, 512:1024],
                                 start=(k == 0), stop=False)
            nc.tensor.matmul(out1_ps[:, 0:512], ones[:, 0:BPC], bow_sb[:, 0:512],
                             start=False, stop=True)
            nc.tensor.matmul(out1_ps[:, 512:1024], ones[:, 0:BPC], bow_sb[:, 512:1024],
                             start=False, stop=True)
            tanh_sb = tail.tile([BPC, H], F32)
            nc.scalar.activation(tanh_sb[:], out1_ps[:], mybir.ActivationFunctionType.Tanh)
            prod = tail.tile([BPC, H], F32)
            nc.vector.tensor_mul(prod[:], tanh_sb[:], wvb_sb[:])
            x0 = tail.tile([BPC, 1], F32)
            nc.vector.reduce_sum(x0[:], prod[:], axis=mybir.AxisListType.X)
            x1 = tail.tile([BPC, 1], F32)
            nc.vector.tensor_add(x1[:], x0[:], bv_sb[:])
            # sigmoid via exp + reciprocal (ACT Sigmoid LUT is too coarse: 40 ULP)
            enx = tail.tile([BPC, 1], F32)
            nc.scalar.activation(enx[:], x1[:], mybir.ActivationFunctionType.Exp,
                                 scale=-1.0)
            den = tail.tile([BPC, 1], F32)
            nc.vector.tensor_scalar_add(den[:], enx[:], 1.0)
            sig = tail.tile([BPC, 1], F32)
            nc.vector.reciprocal(sig[:], den[:])
            pt = tail.tile([BPC, 1], F32)
            nc.scalar.activation(pt[:], sig[:], mybir.ActivationFunctionType.Copy,
                                 bias=float(D), scale=SIG_SCALE)
            # floor(pt): int cast (any rounding) + fixup where result > pt
            base_i = tail.tile([BPC, 1], I32)
            nc.vector.tensor_copy(base_i[:], pt[:])
            base_r = tail.tile([BPC, 1], F32)
            nc.vector.tensor_copy(base_r[:], base_i[:])
            gtm = tail.tile([BPC, 1], F32)
            nc.vector.tensor_tensor(gtm[:], base_r[:], pt[:], op=mybir.AluOpType.is_gt)
            basef = tail.tile([BPC, 1], F32)
            nc.vector.tensor_sub(basef[:], base_r[:], gtm[:])
            frac = tail.tile([BPC, 1], F32)
            nc.vector.tensor_sub(frac[:], pt[:], basef[:])
            flatstart = tail.tile([BPC, 1], F32)
            nc.vector.tensor_add(flatstart[:], basef[:], ro_f[:])
            fsT_ps = psum_tr.tile([1, BPC], F32, tag="tr")
            nc.tensor.transpose(fsT_ps[:], flatstart[:], i128[:BPC, :BPC])
            fsT_sb = tail.tile([1, BPC], F32)
            nc.vector.tensor_copy(fsT_sb[:], fsT_ps[:])
            flatb_ps = psum_tr.tile([128, BPC], F32, tag="tr")
            nc.tensor.matmul(flatb_ps[:], ones[:], fsT_sb[:], start=True, stop=True)
            idxf = tail.tile([128, BPC], F32)
            nc.vector.tensor_scalar(idxf[:], flatb_ps[:], wi_f[:, 0:1], None,
                                    op0=mybir.AluOpType.add)
            idx_all = tail.tile([128, BPC], I32)
            nc.vector.tensor_copy(idx_all[:], idxf[:])
            idxLf = tail.tile([BPC, 1], F32)
            nc.vector.tensor_scalar_add(idxLf[:], flatstart[:], 128.0)
            idxL = tail.tile([BPC, 1], I32)
            nc.vector.tensor_copy(idxL[:], idxLf[:])

            # ---- v = W_attn @ s_b  (v_sb[p, m, b] = v[128m+p, b]), c_b = b_attn.s_b
            v_sb = wpool.tile([128, KC, BPC], F32)
            for m in range(KC):
                v_ps = psum_sm.tile([128, BPC], F32, tag="sm")
                for k in range(KC):
                    nc.tensor.matmul(v_ps[:], wat_sb[:, k, ts(m, 128)], sT_sb[:, k, :],
                                     start=(k == 0), stop=(k == KC - 1))
                nc.vector.tensor_copy(v_sb[:, m, :], v_ps[:])
            c_ps = psum_sm.tile([1, BPC], F32, tag="sm")
            for k in range(KC):
                nc.tensor.matmul(c_ps[:], baT_sb[:, k:k + 1], sT_sb[:, k, :],
                                 start=(k == 0), stop=(k == KC - 1))
            c_sb = tail.tile([1, BPC], F32)
            nc.vector.tensor_copy(c_sb[:], c_ps[:])

            alpha_m = tail.tile([128, BPC], F32)

            # ---- main loop: one 128-row window tile per sample ----
            for b in range(BPC):
                a_t = pool_a.tile([128, H], F32, tag="a")
                nc.gpsimd.indirect_dma_start(
                    out=a_t[:], out_offset=None, in_=enc_h[:, :],
                    in_offset=IndirectOffsetOnAxis(ap=idx_all[:, b:b + 1], axis=0))
                at_t = pool_at.tile([128, KC, 128], F32, tag="at")
                for k in range(KC):
                    tr = psum_tr.tile([128, 128], F32, tag="tr")
                    nc.tensor.transpose(tr[:], a_t[:, ts(k, 128)], i128[:])
                    nc.vector.tensor_copy(at_t[:, k, :], tr[:])
                out_ps = psum_o.tile([128, H], F32, tag="o")
                al_ps = psum_sm.tile([128, 1], F32, tag="sm")
                for k in range(KC):
                    lhsT = at_t[:, k, :]
                    nc.tensor.matmul(out_ps[:, 0:512], lhsT, wa_sb[:, k, 0:512],
                                     start=(k == 0), stop=False)
                    nc.tensor.matmul(out_ps[:, 512:1024], lhsT, wa_sb[:, k, 512:1024],
                                     start=(k == 0), stop=False)
                    nc.tensor.matmul(al_ps[:], lhsT, v_sb[:, k, b:b + 1],
                                     start=(k == 0), stop=False)
                nc.tensor.matmul(out_ps[:, 0:512], ones[:], ba_sb[:, 0:512],
                                 start=False, stop=True)
                nc.tensor.matmul(out_ps[:, 512:1024], ones[:], ba_sb[:, 512:1024],
                                 start=False, stop=True)
                nc.tensor.matmul(al_ps[:], ones[:], c_sb[:, b:b + 1],
                                 start=False, stop=True)
                out_sb = pool_osb.tile([128, H], F32, tag="osb")
                nc.vector.tensor_copy(out_sb[:], out_ps[:])
                nc.sync.dma_start(out=encwt_h[W * b: W * b + 128, :], in_=out_sb[:])
                nc.vector.tensor_copy(alpha_m[:, b:b + 1], al_ps[:])

            # ---- leftover row (w=128) of every sample, as one [16, H] tile ----
            a_l = pool_a.tile([BPC, H], F32, tag="a")
            nc.gpsimd.indirect_dma_start(
                out=a_l[:], out_offset=None, in_=enc_h[:, :],
                in_offset=IndirectOffsetOnAxis(ap=idxL[:, 0:1], axis=0))
            at_l = pool_at.tile([128, KC, BPC], F32, tag="at")
            for k in range(KC):
                tr = psum_tr.tile([128, BPC], F32, tag="tr")
                nc.tensor.transpose(tr[:], a_l[:, ts(k, 128)], i128[:BPC, :BPC])
                nc.vector.tensor_copy(at_l[:, k, :], tr[:])
            outl_ps = psum_o.tile([BPC, H], F32, tag="o")
            all_ps = psum_sm.tile([BPC, BPC], F32, tag="sm")
            for k in range(KC):
                lhsT = at_l[:, k, :]
                nc.tensor.matmul(outl_ps[:, 0:512], lhsT, wa_sb[:, k, 0:512],
                                 start=(k == 0), stop=False)
                nc.tensor.matmul(outl_ps[:, 512:1024], lhsT, wa_sb[:, k, 512:1024],
                                 start=(k == 0), stop=False)
                nc.tensor.matmul(all_ps[:], lhsT, v_sb[:, k, :],
                                 start=(k == 0), stop=False)
            nc.tensor.matmul(outl_ps[:, 0:512], ones[:, 0:BPC], ba_sb[:, 0:512],
                             start=False, stop=True)
            nc.tensor.matmul(outl_ps[:, 512:1024], ones[:, 0:BPC], ba_sb[:, 512:1024],
                             start=False, stop=True)
            nc.tensor.matmul(all_ps[:], ones[:, 0:BPC], c_sb[:], start=False, stop=True)
            outl_sb = pool_osb.tile([BPC, H], F32, tag="osb")
            nc.vector.tensor_copy(outl_sb[:], outl_ps[:])
            nc.sync.dma_start(
                out=bass.AP(tensor=encwt_h[:, :].tensor, offset=encwt_h[:, :].offset + 128 * H,
                            ap=[[W * H, BPC], [1, H]]),
                in_=outl_sb[:])
            alm = tail.tile([BPC, BPC], F32)
            nc.vector.tensor_mul(alm[:], all_ps[:], i128[:BPC, :BPC])
            al_sb = tail.tile([BPC, 1], F32)
            nc.vector.reduce_sum(al_sb[:], alm[:], axis=mybir.AxisListType.X)

            # ---- alpha -> [16, 129], gauss, softmax ----
            aT_ps = psum_tr.tile([BPC, 128], F32, tag="tr")
            nc.tensor.transpose(aT_ps[:], alpha_m[:], i128[:])
            alpha16 = tail.tile([BPC, W], F32)
            nc.vector.tensor_copy(alpha16[:, 0:128], aT_ps[:])
            nc.vector.tensor_copy(alpha16[:, 128:W], al_sb[:])
            targ = tail.tile([BPC, W], F32)
            nc.vector.tensor_scalar(targ[:], wm_f[:], frac[:, 0:1], None,
                                    op0=mybir.AluOpType.subtract)
            t2 = tail.tile([BPC, W], F32)
            nc.vector.tensor_mul(t2[:], targ[:], targ[:])
            gs = tail.tile([BPC, W], F32)
            nc.scalar.activation(gs[:], t2[:], mybir.ActivationFunctionType.Exp,
                                 scale=-1.0 / (2.0 * (D / 2.0) ** 2))
            en = tail.tile([BPC, W], F32)
            nc.vector.tensor_mul(en[:], alpha16[:], gs[:])
            mx = tail.tile([BPC, 1], F32)
            nc.vector.reduce_max(mx[:], en[:], axis=mybir.AxisListType.X)
            nmx = tail.tile([BPC, 1], F32)
            nc.vector.tensor_scalar_mul(nmx[:], mx[:], -1.0)
            pex = tail.tile([BPC, W], F32)
            ssum = tail.tile([BPC, 1], F32)
            nc.scalar.activation(pex[:], en[:], mybir.ActivationFunctionType.Exp,
                                 bias=nmx[:, 0:1], accum_out=ssum[:])
            rec = tail.tile([BPC, 1], F32)
            nc.vector.reciprocal(rec[:], ssum[:])
            soft_sb = tail.tile([BPC, W], F32)
            nc.vector.tensor_scalar_mul(soft_sb[:], pex[:], rec[:, 0:1])
            nc.sync.dma_start(out=soft_h[:, :], in_=soft_sb[:])

    return nc


def _in_maps(inputs):
    enc = np.ascontiguousarray(np.asarray(inputs["enc_output"], dtype=np.float32))
    s = np.ascontiguousarray(np.asarray(inputs["s"], dtype=np.float32))
    wow = np.ascontiguousarray(np.asarray(inputs["W_omega_w"], dtype=np.float32))
    bow = np.ascontiguousarray(np.asarray(inputs["b_omega_w"], dtype=np.float32))
    wv = np.ascontiguousarray(np.asarray(inputs["W_omega_v"], dtype=np.float32)[:, 0])
    bv = np.ascontiguousarray(np.asarray(inputs["b_omega_v"], dtype=np.float32))
    wa = np.ascontiguousarray(np.asarray(inputs["W_attn"], dtype=np.float32))
    wat = np.ascontiguousarray(wa.T)
    ba = np.ascontiguousarray(np.asarray(inputs["b_attn"], dtype=np.float32))
    maps = []
    for c in range(NC):
        sl = slice(c * BPC, (c + 1) * BPC)
        maps.append({
            "enc": enc[sl].reshape(BPC * S_LEN, H),
            "st": np.ascontiguousarray(s[sl].T),
            "wow": wow, "bow": bow, "wv": wv, "bv": bv,
            "wa": wa, "wat": wat, "ba": ba,
        })
    return maps


def _run(inputs, trace=False):
    if "nc" not in _CACHE:
        nc = _build()
        nc.finalize()
        _CACHE["nc"] = nc
    nc = _CACHE["nc"]
    res = run_bass_kernel_spmd(nc, _in_maps(inputs), list(range(NC)), trace=trace)
    soft = np.concatenate([r["soft"] for r in res.results], axis=0)
    encwt = np.concatenate(
        [r["encwt"].reshape(BPC, W, H) for r in res.results], axis=0)
    return (soft, encwt), res


def kernel(**inputs):
    out, _ = _run(inputs, trace=False)
    return out


def kernel_traced(**inputs):
    out, res = _run(inputs, trace=True)
    return out, res


# revision 7
# speedup vs baseline: 1.0001x; 1.0001x over previous
"""Trainium2 Bass kernel for nn_LocalAttention (sparse window attention).

Math (per sample b):
  x_b      = tanh(s_b @ W_omega_w + b_omega_w) @ W_omega_v + b_omega_v
  pt_b     = 1919 * sigmoid(x_b) + 64
  base_b   = floor(pt_b)
  idx      = base_b + [-64..64]                       (129 window rows)
  enc_w    = enc_output[b, idx, :]                    (gathered on device)
  enc_w_t  = enc_w @ W_attn + b_attn                  -> output 2
  alpha    = enc_w_t . s_b  (== enc_w . (W_attn@s_b) + b_attn.s_b)
  gauss    = exp(-((w-64-frac_b)^2)/2048)
  out1     = softmax(alpha * gauss)                   -> output 1

Sharding: data-parallel over batch; 16 samples per core on 8 cores.
The window rows are fetched with indirect (gather) DMA so only ~67MB of
enc_output is ever read instead of the full 1GB.
"""

import numpy as np

import concourse.bass as bass
import concourse.bacc as bacc
import concourse.mybir as mybir
import concourse.tile as tile
from concourse.bass import IndirectOffsetOnAxis, ts
from concourse.bass_utils import run_bass_kernel_spmd
from concourse.masks import make_identity

F32 = mybir.dt.float32
I32 = mybir.dt.int32

B = 128
NC = 8
BPC = B // NC          # 16 samples per core
S_LEN = 2048
H = 1024
D = 64
W = 2 * D + 1          # 129
KC = H // 128          # 8 contraction chunks
SIG_SCALE = float(S_LEN - 2 * D - 1)  # 1919

_CACHE = {}


def _build():
    nc = bacc.Bacc()
    enc_h = nc.declare_dram_parameter("enc", [BPC * S_LEN, H], F32, isOutput=False)
    sT_h = nc.declare_dram_parameter("st", [H, BPC], F32, isOutput=False)
    wow_h = nc.declare_dram_parameter("wow", [H, H], F32, isOutput=False)
    bow_h = nc.declare_dram_parameter("bow", [H], F32, isOutput=False)
    wv_h = nc.declare_dram_parameter("wv", [H], F32, isOutput=False)
    bv_h = nc.declare_dram_parameter("bv", [1], F32, isOutput=False)
    wa_h = nc.declare_dram_parameter("wa", [H, H], F32, isOutput=False)
    wat_h = nc.declare_dram_parameter("wat", [H, H], F32, isOutput=False)
    ba_h = nc.declare_dram_parameter("ba", [H], F32, isOutput=False)
    encwt_h = nc.declare_dram_parameter("encwt", [BPC * W, H], F32, isOutput=True)
    soft_h = nc.declare_dram_parameter("soft", [BPC, W], F32, isOutput=True)

    from contextlib import ExitStack

    with tile.TileContext(nc) as tc, ExitStack() as ctx:
        consts = ctx.enter_context(tc.tile_pool(name="consts", bufs=1))
        wpool = ctx.enter_context(tc.tile_pool(name="weights", bufs=1))
        pool_a = ctx.enter_context(tc.tile_pool(name="gather", bufs=12))
        pool_at = ctx.enter_context(tc.tile_pool(name="at", bufs=2))
        pool_osb = ctx.enter_context(tc.tile_pool(name="osb", bufs=3))
        tail = ctx.enter_context(tc.tile_pool(name="tail", bufs=1))
        psum_o = ctx.enter_context(tc.tile_pool(name="pso", bufs=2, space="PSUM"))
        psum_tr = ctx.enter_context(tc.tile_pool(name="pstr", bufs=2, space="PSUM"))
        psum_sm = ctx.enter_context(tc.tile_pool(name="pssm", bufs=2, space="PSUM"))

        if True:
            # ---- constants (on-device) ----
            i128 = consts.tile([128, 128], F32)
            make_identity(nc, i128[:])
            ones = consts.tile([1, 128], F32)
            nc.vector.memset(ones[:], 1.0)
            ro_i = consts.tile([BPC, 1], I32)
            nc.gpsimd.iota(ro_i[:], pattern=[[0, 1]], base=-D,
                           channel_multiplier=S_LEN)
            ro_f = consts.tile([BPC, 1], F32)
            nc.vector.tensor_copy(ro_f[:], ro_i[:])
            wi_i = consts.tile([128, 1], I32)
            nc.gpsimd.iota(wi_i[:], pattern=[[0, 1]], base=0, channel_multiplier=1)
            wi_f = consts.tile([128, 1], F32)
            nc.vector.tensor_copy(wi_f[:], wi_i[:])
            wm_i = consts.tile([BPC, W], I32)
            nc.gpsimd.iota(wm_i[:], pattern=[[1, W]], base=-D, channel_multiplier=0)
            wm_f = consts.tile([BPC, W], F32)
            nc.vector.tensor_copy(wm_f[:], wm_i[:])

            # ---- resident weights ----
            wa_sb = wpool.tile([128, KC, H], F32)
            nc.sync.dma_start(out=wa_sb[:], in_=wa_h[:, :].rearrange("(k p) n -> p k n", p=128))
            wat_sb = wpool.tile([128, KC, H], F32)
            nc.sync.dma_start(out=wat_sb[:], in_=wat_h[:, :].rearrange("(k p) n -> p k n", p=128))
            wow_sb = wpool.tile([128, KC, H], F32)
            nc.sync.dma_start(out=wow_sb[:], in_=wow_h[:, :].rearrange("(k p) n -> p k n", p=128))
            sT_sb = wpool.tile([128, KC, BPC], F32)
            nc.sync.dma_start(out=sT_sb[:], in_=sT_h[:, :].rearrange("(k p) b -> p k b", p=128))
            ba_sb = wpool.tile([1, H], F32)
            nc.sync.dma_start(out=ba_sb[:], in_=ba_h[:].rearrange("(o n) -> o n", o=1))
            baT_sb = wpool.tile([128, KC], F32)
            nc.sync.dma_start(out=baT_sb[:], in_=ba_h[:].rearrange("(k p) -> p k", p=128))
            bow_sb = wpool.tile([1, H], F32)
            nc.sync.dma_start(out=bow_sb[:], in_=bow_h[:].rearrange("(o n) -> o n", o=1))
            wvb_sb = wpool.tile([BPC, H], F32)
            nc.sync.dma_start(out=wvb_sb[:], in_=bass.AP(
                tensor=wv_h[:].tensor, offset=wv_h[:].offset, ap=[[0, BPC], [1, H]]))
            bv_sb = wpool.tile([BPC, 1], F32)
            nc.sync.dma_start(out=bv_sb[:], in_=bass.AP(
                tensor=bv_h[:].tensor, offset=bv_h[:].offset, ap=[[0, BPC], [1, 1]]))

            # ---- stage A: pt / base / gather indices ----
            out1_ps = psum_o.tile([BPC, H], F32, tag="o")
            for k in range(KC):
                lhsT = sT_sb[:, k, :]
                nc.tensor.matmul(out1_ps[:, 0:512], lhsT, wow_sb[:, k, 0:512],
                                 start=(k == 0), stop=False)
                nc.tensor.matmul(out1_ps[:, 512:1024], lhsT, wow_sb[:, k, 512:1024],
                                 start=(k == 0), stop=False)
            nc.tensor.matmul(out1_ps[:, 0:512], ones[:, 0:BPC], bow_sb[:, 0:512],
                             start=False, stop=True)
            nc.tensor.matmul(out1_ps[:, 512:1024], ones[:, 0:BPC], bow_sb[:, 512:1024],
                             start=False, stop=True)
            tanh_sb = tail.tile([BPC, H], F32)
            nc.scalar.activation(tanh_sb[:], out1_ps[:], mybir.ActivationFunctionType.Tanh)
            prod = tail.tile([BPC, H], F32)
            nc.vector.tensor_mul(prod[:], tanh_sb[:], wvb_sb[:])
            x0 = tail.tile([BPC, 1], F32)
            nc.vector.reduce_sum(x0[:], prod[:], axis=mybir.AxisListType.X)
            x1 = tail.tile([BPC, 1], F32)
            nc.vector.tensor_add(x1[:], x0[:], bv_sb[:])
            # sigmoid via exp + reciprocal (ACT Sigmoid LUT is too coarse: 40 ULP)
            enx = tail.tile([BPC, 1], F32)
            nc.scalar.activation(enx[:], x1[:], mybir.ActivationFunctionType.Exp,
                                 scale=-1.0)
            den = tail.tile([BPC, 1], F32)
            nc.vector.tensor_scalar_add(den[:], enx[:], 1.0)
            sig = tail.tile([BPC, 1], F32)
            nc.vector.reciprocal(sig[:], den[:])
            pt = tail.tile([BPC, 1], F32)
            nc.scalar.activation(pt[:], sig[:], mybir.ActivationFunctionType.Copy,
                                 bias=float(D), scale=SIG_SCALE)
            # floor(pt): int cast (any rounding) + fixup where result > pt
            base_i = tail.tile([BPC, 1], I32)
            nc.vector.tensor_copy(base_i[:], pt[:])
            base_r = tail.tile([BPC, 1], F32)
            nc.vector.tensor_copy(base_r[:], base_i[:])
            gtm = tail.tile([BPC, 1], F32)
            nc.vector.tensor_tensor(gtm[:], base_r[:], pt[:], op=mybir.AluOpType.is_gt)
            basef = tail.tile([BPC, 1], F32)
            nc.vector.tensor_sub(basef[:], base_r[:], gtm[:])
            frac = tail.tile([BPC, 1], F32)
            nc.vector.tensor_sub(frac[:], pt[:], basef[:])
            flatstart = tail.tile([BPC, 1], F32)
            nc.vector.tensor_add(flatstart[:], basef[:], ro_f[:])
            fsT_ps = psum_tr.tile([1, BPC], F32, tag="tr")
            nc.tensor.transpose(fsT_ps[:], flatstart[:], i128[:BPC, :BPC])
            fsT_sb = tail.tile([1, BPC], F32)
            nc.vector.tensor_copy(fsT_sb[:], fsT_ps[:])
            flatb_ps = psum_tr.tile([128, BPC], F32, tag="tr")
            nc.tensor.matmul(flatb_ps[:], ones[:], fsT_sb[:], start=True, stop=True)
            idxf = tail.tile([128, BPC], F32)
            nc.vector.tensor_scalar(idxf[:], flatb_ps[:], wi_f[:, 0:1], None,
                                    op0=mybir.AluOpType.add)
            idx_all = tail.tile([128, BPC], I32)
            nc.vector.tensor_copy(idx_all[:], idxf[:])
            idxLf = tail.tile([BPC, 1], F32)
            nc.vector.tensor_scalar_add(idxLf[:], flatstart[:], 128.0)
            idxL = tail.tile([BPC, 1], I32)
            nc.vector.tensor_copy(idxL[:], idxLf[:])

            # ---- v = W_attn @ s_b  (v_sb[p, m, b] = v[128m+p, b]), c_b = b_attn.s_b
            v_sb = wpool.tile([128, KC, BPC], F32)
            for m in range(KC):
                v_ps = psum_sm.tile([128, BPC], F32, tag="sm")
                for k in range(KC):
                    nc.tensor.matmul(v_ps[:], wat_sb[:, k, ts(m, 128)], sT_sb[:, k, :],
                                     start=(k == 0), stop=(k == KC - 1))
                nc.vector.tensor_copy(v_sb[:, m, :], v_ps[:])
            c_ps = psum_sm.tile([1, BPC], F32, tag="sm")
            for k in range(KC):
                nc.tensor.matmul(c_ps[:], baT_sb[:, k:k + 1], sT_sb[:, k, :],
                                 start=(k == 0), stop=(k == KC - 1))
            c_sb = tail.tile([1, BPC], F32)
            nc.vector.tensor_copy(c_sb[:], c_ps[:])

            alpha_m = tail.tile([128, BPC], F32)

            # ---- pre-issue every gather so PE work stays dense ----
            a_tiles = []
            for b in range(BPC):
                a_t = pool_a.tile([128, H], F32, tag="a")
                nc.gpsimd.indirect_dma_start(
                    out=a_t[:], out_offset=None, in_=enc_h[:, :],
                    in_offset=IndirectOffsetOnAxis(ap=idx_all[:, b:b + 1], axis=0))
                a_tiles.append(a_t)

            # ---- main loop: one 128-row window tile per sample ----
            for b in range(BPC):
                a_t = a_tiles[b]
                at_t = pool_at.tile([128, KC, 128], F32, tag="at")
                for k in range(KC):
                    tr = psum_tr.tile([128, 128], F32, tag="tr")
                    nc.tensor.transpose(tr[:], a_t[:, ts(k, 128)], i128[:])
                    nc.vector.tensor_copy(at_t[:, k, :], tr[:])
                out_ps = psum_o.tile([128, H], F32, tag="o")
                al_ps = psum_sm.tile([128, 1], F32, tag="sm")
                for k in range(KC):
                    lhsT = at_t[:, k, :]
                    nc.tensor.matmul(out_ps[:, 0:512], lhsT, wa_sb[:, k, 0:512],
                                     start=(k == 0), stop=False)
                    nc.tensor.matmul(out_ps[:, 512:1024], lhsT, wa_sb[:, k, 512:1024],
                                     start=(k == 0), stop=False)
                    nc.tensor.matmul(al_ps[:], lhsT, v_sb[:, k, b:b + 1],
                                     start=(k == 0), stop=False)
                nc.tensor.matmul(out_ps[:, 0:512], ones[:], ba_sb[:, 0:512],
                                 start=False, stop=True)
                nc.tensor.matmul(out_ps[:, 512:1024], ones[:], ba_sb[:, 512:1024],
                                 start=False, stop=True)
                nc.tensor.matmul(al_ps[:], ones[:], c_sb[:, b:b + 1],
                                 start=False, stop=True)
                out_sb = pool_osb.tile([128, H], F32, tag="osb")
                nc.vector.tensor_copy(out_sb[:], out_ps[:])
                nc.sync.dma_start(out=encwt_h[W * b: W * b + 128, :], in_=out_sb[:])
                nc.vector.tensor_copy(alpha_m[:, b:b + 1], al_ps[:])

            # ---- leftover row (w=128) of every sample, as one [16, H] tile ----
            a_l = pool_a.tile([BPC, H], F32, tag="a")
            nc.gpsimd.indirect_dma_start(
                out=a_l[:], out_offset=None, in_=enc_h[:, :],
                in_offset=IndirectOffsetOnAxis(ap=idxL[:, 0:1], axis=0))
            at_l = pool_at.tile([128, KC, BPC], F32, tag="at")
            for k in range(KC):
                tr = psum_tr.tile([128, BPC], F32, tag="tr")
                nc.tensor.transpose(tr[:], a_l[:, ts(k, 128)], i128[:BPC, :BPC])
                nc.vector.tensor_copy(at_l[:, k, :], tr[:])
            outl_ps = psum_o.tile([BPC, H], F32, tag="o")
            all_ps = psum_sm.tile([BPC, BPC], F32, tag="sm")
            for k in range(KC):
                lhsT = at_l[:, k, :]
                nc.tensor.matmul(outl_ps[:, 0:512], lhsT, wa_sb[:, k, 0:512],
                                 start=(k == 0), stop=False)
                nc.tensor.matmul(outl_ps[:, 512:1024], lhsT, wa_sb[:, k, 512:1024],
                                 start=(k == 0), stop=False)
                nc.tensor.matmul(all_ps[:], lhsT, v_sb[:, k, :],
                                 start=(k == 0), stop=False)
            nc.tensor.matmul(outl_ps[:, 0:512], ones[:, 0:BPC], ba_sb[:, 0:512],
                             start=False, stop=True)
            nc.tensor.matmul(outl_ps[:, 512:1024], ones[:, 0:BPC], ba_sb[:, 512:1024],
                             start=False, stop=True)
            nc.tensor.matmul(all_ps[:], ones[:, 0:BPC], c_sb[:], start=False, stop=True)
            outl_sb = pool_osb.tile([BPC, H], F32, tag="osb")
            nc.vector.tensor_copy(outl_sb[:], outl_ps[:])
            nc.sync.dma_start(
                out=bass.AP(tensor=encwt_h[:, :].tensor, offset=encwt_h[:, :].offset + 128 * H,
                            ap=[[W * H, BPC], [1, H]]),
                in_=outl_sb[:])
            alm = tail.tile([BPC, BPC], F32)
            nc.vector.tensor_mul(alm[:], all_ps[:], i128[:BPC, :BPC])
            al_sb = tail.tile([BPC, 1], F32)
            nc.vector.reduce_sum(al_sb[:], alm[:], axis=mybir.AxisListType.X)

            # ---- alpha -> [16, 129], gauss, softmax ----
            aT_ps = psum_tr.tile([BPC, 128], F32, tag="tr")
            nc.tensor.transpose(aT_ps[:], alpha_m[:], i128[:])
            alpha16 = tail.tile([BPC, W], F32)
            nc.vector.tensor_copy(alpha16[:, 0:128], aT_ps[:])
            nc.vector.tensor_copy(alpha16[:, 128:W], al_sb[:])
            targ = tail.tile([BPC, W], F32)
            nc.vector.tensor_scalar(targ[:], wm_f[:], frac[:, 0:1], None,
                                    op0=mybir.AluOpType.subtract)
            t2 = tail.tile([BPC, W], F32)
            nc.vector.tensor_mul(t2[:], targ[:], targ[:])
            gs = tail.tile([BPC, W], F32)
            nc.scalar.activation(gs[:], t2[:], mybir.ActivationFunctionType.Exp,
                                 scale=-1.0 / (2.0 * (D / 2.0) ** 2))
            en = tail.tile([BPC, W], F32)
            nc.vector.tensor_mul(en[:], alpha16[:], gs[:])
            mx = tail.tile([BPC, 1], F32)
            nc.vector.reduce_max(mx[:], en[:], axis=mybir.AxisListType.X)
            nmx = tail.tile([BPC, 1], F32)
            nc.vector.tensor_scalar_mul(nmx[:], mx[:], -1.0)
            pex = tail.tile([BPC, W], F32)
            ssum = tail.tile([BPC, 1], F32)
            nc.scalar.activation(pex[:], en[:], mybir.ActivationFunctionType.Exp,
                                 bias=nmx[:, 0:1], accum_out=ssum[:])
            rec = tail.tile([BPC, 1], F32)
            nc.vector.reciprocal(rec[:], ssum[:])
            soft_sb = tail.tile([BPC, W], F32)
            nc.vector.tensor_scalar_mul(soft_sb[:], pex[:], rec[:, 0:1])
            nc.sync.dma_start(out=soft_h[:, :], in_=soft_sb[:])

    return nc


def _in_maps(inputs):
    enc = np.ascontiguousarray(np.asarray(inputs["enc_output"], dtype=np.float32))
    s = np.ascontiguousarray(np.asarray(inputs["s"], dtype=np.float32))
    wow = np.ascontiguousarray(np.asarray(inputs["W_omega_w"], dtype=np.float32))
    bow = np.ascontiguousarray(np.asarray(inputs["b_omega_w"], dtype=np.float32))
    wv = np.ascontiguousarray(np.asarray(inputs["W_omega_v"], dtype=np.float32)[:, 0])
    bv = np.ascontiguousarray(np.asarray(inputs["b_omega_v"], dtype=np.float32))
    wa = np.ascontiguousarray(np.asarray(inputs["W_attn"], dtype=np.float32))
    wat = np.ascontiguousarray(wa.T)
    ba = np.ascontiguousarray(np.asarray(inputs["b_attn"], dtype=np.float32))
    maps = []
    for c in range(NC):
        sl = slice(c * BPC, (c + 1) * BPC)
        maps.append({
            "enc": enc[sl].reshape(BPC * S_LEN, H),
            "st": np.ascontiguousarray(s[sl].T),
            "wow": wow, "bow": bow, "wv": wv, "bv": bv,
            "wa": wa, "wat": wat, "ba": ba,
        })
    return maps


def _run(inputs, trace=False):
    if "nc" not in _CACHE:
        nc = _build()
        nc.finalize()
        _CACHE["nc"] = nc
    nc = _CACHE["nc"]
    res = run_bass_kernel_spmd(nc, _in_maps(inputs), list(range(NC)), trace=trace)
    soft = np.concatenate([r["soft"] for r in res.results], axis=0)
    encwt = np.concatenate(
        [r["encwt"].reshape(BPC, W, H) for r in res.results], axis=0)
    return (soft, encwt), res


def kernel(**inputs):
    out, _ = _run(inputs, trace=False)
    return out


def kernel_traced(**inputs):
    out, res = _run(inputs, trace=True)
    return out, res
